# revision 27
# baseline (speedup 1.0000x reference)
"""Trainium2 Bass kernel for nn_AffinityDiffProposedModel (v2).

Reference model (B=4, L=256, D=512, H=8, DH=64):
  Q/K/V = relu(x @ W + b); euclidean diff-attention per head
  (logits = -||q-k||/sqrt(DH)), softmax over keys, query-mask,
  ctx @ W_bil @ keys^T + b_bil -> (B, L, L).

Sharding: 8 cores = 4 batches x 2 query-halves (128 query rows each).
Each core computes its (128, 256) slice of the output.

v2 design notes:
  * All matmul operands are bf16 (1 row/cycle at any free size, half the
    HBM traffic); PSUM accumulates in f32. rel-err gate is 2e-2; this
    lands ~4e-3.
  * Host packs transposed inputs (qsT/kbT/vbT) and chunk-major weights,
    so no PE transposes are needed on the way in. The query mask is a
    host-computed column.
  * The distance matrix is built TRANSPOSED, d2T[k, q], so exp() writes
    attn^T directly and the per-head attention needs no transposes:
      d2T = -2*(qk^T - k2[k]/2 - q2[q]/2)
    with the k2/q2 terms folded into the same PSUM accumulation group as
    K=64 matmuls against a constant -0.5 operand (the all-constant side
    broadcasts the contraction of the other side). Sqrt then needs no
    bias and runs per-head; exactly 2 ACT table loads (Sqrt, Exp) total.
  * HW constraint (found by probing): matmuls whose stationary operands
    sit at different partition bases (head-even at 0, head-odd at 64)
    fault the NEFF if they target the same PSUM bank -> each head's d2T
    gets its own PSUM tile.
  * Softmax normalization is deferred: ctx_un = pT^T @ [V | 1] puts the
    softmax row-sum s in PSUM column 64; ctx = ctx_un * (1/s) fuses into
    the per-head PSUM->SBUF copy. Query-mask and b_bil fuse into the
    final output copy.
  * Bilinear tail: ctx -> ctxT via 4 PE transposes; interT computed
    directly per e-chunk as Wb^T-stationary matmuls (no inter transpose).
"""

import os
import sys

import numpy as np

B, L, D, H = 4, 256, 512, 8
DH = 64
LQ = 128  # query rows per core
NC = 4  # D // 128 feature chunks
NR = 2  # L // 128 key-row chunks
N_CORES = 8

_REPO = "/opt/trn_rl_repo"


def _ensure_path():
    if _REPO not in sys.path:
        sys.path.insert(0, _REPO)


def build_nc():
    _ensure_path()
    import concourse.bacc as bacc
    import concourse.mybir as mybir
    import concourse.tile as tile

    nc = bacc.Bacc("TRN2", target_bir_lowering=False, debug=False, num_devices=N_CORES)

    f32 = mybir.dt.float32
    bf16 = mybir.dt.bfloat16

    # ---- DRAM I/O ----
    qsT = nc.dram_tensor("qsT", [128, NC, LQ], bf16, kind="ExternalInput").ap()
    kbT = nc.dram_tensor("kbT", [128, NC, L], bf16, kind="ExternalInput").ap()
    vbT = nc.dram_tensor("vbT", [128, NC, L], bf16, kind="ExternalInput").ap()
    Wq = nc.dram_tensor("Wq", [128, NC, D], bf16, kind="ExternalInput").ap()
    Wk = nc.dram_tensor("Wk", [128, NC, D], bf16, kind="ExternalInput").ap()
    Wv = nc.dram_tensor("Wv", [128, NC, D], bf16, kind="ExternalInput").ap()
    Wb = nc.dram_tensor("Wb", [128, NC, D], bf16, kind="ExternalInput").ap()
    # f32 consts packed [128, 2*NC+2]: bqT | bkT | qm | bbil
    cst = nc.dram_tensor("cst", [128, 2 * NC + 2], f32, kind="ExternalInput").ap()
    # bf16 rows packed [1, 128 + D]: ones | bv
    rows = nc.dram_tensor("rows", [1, 128 + D], bf16, kind="ExternalInput").ap()
    ident = nc.dram_tensor("ident", [128, 128], bf16, kind="ExternalInput").ap()
    out = nc.dram_tensor("out", [LQ, L], f32, kind="ExternalOutput").ap()

    with tile.TileContext(nc) as tc:
        _body(nc, tc, mybir,
              qsT, kbT, vbT, Wq, Wk, Wv, Wb, cst, rows, ident, out)
    nc.compile()
    return nc


def _body(nc, tc, mybir,
          qsT, kbT, vbT, Wq, Wk, Wv, Wb, cst, rows, ident, out):
    from contextlib import ExitStack

    f32 = mybir.dt.float32
    bf16 = mybir.dt.bfloat16
    Alu = mybir.AluOpType
    Act = mybir.ActivationFunctionType

    ctx = ExitStack()
    with ctx:
        const = ctx.enter_context(tc.tile_pool(name="const", bufs=1))
        persist = ctx.enter_context(tc.tile_pool(name="persist", bufs=1))
        dists = ctx.enter_context(tc.tile_pool(name="dists", bufs=8))
        ps_proj = ctx.enter_context(tc.tile_pool(name="ps_proj", bufs=2, space="PSUM"))
        ps_pair = ctx.enter_context(tc.tile_pool(name="ps_pair", bufs=2, space="PSUM"))
        ps_odd = ctx.enter_context(tc.tile_pool(name="ps_odd", bufs=1, space="PSUM"))
        ps_ctx = ctx.enter_context(tc.tile_pool(name="ps_ctx", bufs=2, space="PSUM"))
        ps_tp = ctx.enter_context(tc.tile_pool(name="ps_tp", bufs=1, space="PSUM"))

        # ---- input loads first, issue split across both HWDGE queues ----
        kbT_sb = persist.tile([128, NC, L], bf16, tag="kbT")
        nc.sync.dma_start(out=kbT_sb, in_=kbT)
        Wk_t = persist.tile([128, NC, D], bf16, tag="wk")
        nc.scalar.dma_start(out=Wk_t, in_=Wk)
        qsT_sb = persist.tile([128, NC, LQ], bf16, tag="qsT")
        nc.sync.dma_start(out=qsT_sb, in_=qsT)
        Wq_t = persist.tile([128, NC, D], bf16, tag="wq")
        nc.scalar.dma_start(out=Wq_t, in_=Wq)
        vbT_sb = persist.tile([128, NC, L], bf16, tag="vbT")
        nc.sync.dma_start(out=vbT_sb, in_=vbT)
        Wv_t = persist.tile([128, NC, D], bf16, tag="wv")
        nc.scalar.dma_start(out=Wv_t, in_=Wv)
        Wb_t = persist.tile([128, NC, D], bf16, tag="wb")
        nc.sync.dma_start(out=Wb_t, in_=Wb)
        Wk_sb = [Wk_t[:, kc, :] for kc in range(NC)]
        Wq_sb = [Wq_t[:, kc, :] for kc in range(NC)]
        Wv_sb = [Wv_t[:, kc, :] for kc in range(NC)]
        Wb_sb = [Wb_t[:, kc, :] for kc in range(NC)]

        # ---- consts (tiny, off the critical path) ----
        cst_sb = const.tile([128, 2 * NC + 2], f32, tag="cst")
        nc.scalar.dma_start(out=cst_sb, in_=cst)
        rows_sb = const.tile([1, 128 + D], bf16, tag="rows")
        nc.sync.dma_start(out=rows_sb, in_=rows)
        ident_sb = const.tile([128, 128], bf16, tag="ident")
        nc.scalar.dma_start(out=ident_sb, in_=ident)
        bqT_sb = cst_sb[:, 0:NC]
        bkT_sb = cst_sb[:, NC:2 * NC]
        qm_sb = cst_sb[:, 2 * NC:2 * NC + 1]
        bbil_sb = cst_sb[:, 2 * NC + 1:2 * NC + 2]
        ones_sb = rows_sb[0:1, 0:128]
        bv_sb = rows_sb[0:1, 128:128 + D]

        # ---- persistent compute tiles ----
        KT = persist.tile([128, NC, L], bf16, tag="KT")
        QT = persist.tile([128, NC, LQ], bf16, tag="QT")
        sqk = persist.tile([128, NC, L], bf16, tag="sqk")
        sq = persist.tile([128, NC, LQ], bf16, tag="sq")
        nhalf = persist.tile([128, L], bf16, tag="nhalf")
        pT_all = persist.tile([128, H, NR, LQ], bf16, tag="pT_all")
        Vaug = persist.tile([128, NR, H, DH + 1], bf16, tag="vaug")
        ctxN = persist.tile([128, D], bf16, tag="ctxN")
        ctxT = persist.tile([128, NC, LQ], bf16, tag="ctxT")
        interT = persist.tile([128, NC, LQ], bf16, tag="interT")
        rs = persist.tile([128, H], f32, tag="rs")
        out_sb = persist.tile([128, L], f32, tag="out_sb")

        nc.gpsimd.memset(Vaug[:, :, :, DH:DH + 1], 1.0)
        nc.gpsimd.memset(nhalf, -0.5)

        # ---- per-chunk: K/Q projections, squares, transposed dist^2 ----
        dist_tiles = []
        for c in range(NC):
            cs = slice(c * 128, (c + 1) * 128)
            # K projection (transposed layout), bias+relu fused on DVE
            pk_t = ps_proj.tile([128, D], f32, tag="proj")
            pk = pk_t[:, 0:L]
            for kc in range(NC):
                nc.tensor.matmul(pk, Wk_sb[kc][:, cs], kbT_sb[:, kc, :],
                                 start=(kc == 0), stop=(kc == NC - 1))
            nc.vector.tensor_scalar(out=KT[:, c, :], in0=pk,
                                    scalar1=bkT_sb[:, c:c + 1], scalar2=0.0,
                                    op0=Alu.add, op1=Alu.max)
            nc.gpsimd.tensor_mul(sqk[:, c, :], KT[:, c, :], KT[:, c, :])
            # Q projection
            pq_t = ps_proj.tile([128, D], f32, tag="proj")
            pq = pq_t[:, 0:LQ]
            for kc in range(NC):
                nc.tensor.matmul(pq, Wq_sb[kc][:, cs], qsT_sb[:, kc, :],
                                 start=(kc == 0), stop=(kc == NC - 1))
            nc.vector.tensor_scalar(out=QT[:, c, :], in0=pq,
                                    scalar1=bqT_sb[:, c:c + 1], scalar2=0.0,
                                    op0=Alu.add, op1=Alu.max)
            nc.gpsimd.tensor_mul(sq[:, c, :], QT[:, c, :], QT[:, c, :])
            # transposed dist^2 per head; separate PSUM tile per head
            # (different stationary partition bases must not share a bank)
            for j in range(2):
                hs = slice(64 * j, 64 * j + 64)
                dpool = ps_pair if j == 0 else ps_odd
                d2 = dpool.tile([128, NR, LQ], f32, tag=f"d2{j}")
                for rc in range(NR):
                    rcs = slice(rc * 128, (rc + 1) * 128)
                    nc.tensor.matmul(d2[:, rc, :], KT[hs, c, rcs],
                                     QT[hs, c, :], start=True, stop=False)
                    nc.tensor.matmul(d2[:, rc, :], sqk[hs, c, rcs],
                                     nhalf[hs, 0:LQ], start=False, stop=False)
                    nc.tensor.matmul(d2[:, rc, :], nhalf[hs, 0:128],
                                     sq[hs, c, :], start=False, stop=True)
                dt_ = dists.tile([128, NR * LQ], bf16, tag="dist")
                nc.scalar.activation(out=dt_,
                                     in_=d2.rearrange("p a b -> p (a b)"),
                                     func=Act.Sqrt, scale=-2.0)
                dist_tiles.append(dt_)

        # ---- V projection (natural layout, bias rides as K=1 matmul) ----
        for rc in range(NR):
            rcs = slice(rc * 128, (rc + 1) * 128)
            pv = ps_proj.tile([128, D], f32, tag="proj")
            for kc in range(NC):
                nc.tensor.matmul(pv, vbT_sb[:, kc, rcs], Wv_sb[kc],
                                 start=(kc == 0), stop=False)
            nc.tensor.matmul(pv, ones_sb, bv_sb, start=False, stop=True)
            nc.vector.tensor_scalar(out=Vaug[:, rc, :, 0:DH],
                                    in0=pv.rearrange("p (h e) -> p h e", h=H),
                                    scalar1=0.0, scalar2=None, op0=Alu.max)

        # ---- exp (per head; single Exp table load for all) ----
        for h in range(H):
            pslice = pT_all[:, h, :, :]
            nc.scalar.activation(out=pslice.rearrange("p a b -> p (a b)"),
                                 in_=dist_tiles[h], func=Act.Exp, scale=-0.125)

        # ---- per-head ctx (+ row-sum via augmented ones column) ----
        for hp in range(H // 2):
            cps2 = ps_ctx.tile([128, 2, DH + 1], f32, tag="cps")
            for i in range(2):
                h = 2 * hp + i
                cps = cps2[:, i, :]
                for rc in range(NR):
                    nc.tensor.matmul(cps, pT_all[:, h, rc, :],
                                     Vaug[:, rc, h, :],
                                     start=(rc == 0), stop=(rc == NR - 1))
                nc.vector.reciprocal(out=rs[:, h:h + 1], in_=cps[:, DH:DH + 1])
                nc.vector.tensor_scalar(out=ctxN[:, h * DH:(h + 1) * DH],
                                        in0=cps[:, 0:DH],
                                        scalar1=rs[:, h:h + 1],
                                        scalar2=None, op0=Alu.mult)

        # ---- bilinear tail ----
        # ctx -> ctxT via PE transposes (one PSUM tile, one copy)
        tp = ps_tp.tile([128, NC, LQ], bf16, tag="tp")
        for dc in range(NC):
            nc.tensor.transpose(tp[:, dc, :], ctxN[:, dc * 128:(dc + 1) * 128],
                                ident_sb)
        nc.vector.tensor_copy(out=ctxT, in_=tp)
        # interT[e, q] = sum_d Wb[d, e] * ctxT[d, q], per 128-wide e-chunk
        for ec in range(NC):
            ecs = slice(ec * 128, (ec + 1) * 128)
            it_t = ps_pair.tile([128, NR, LQ], f32, tag="d20")
            it = it_t[:, 0, :]
            for dc in range(NC):
                nc.tensor.matmul(it, Wb_sb[dc][:, ecs], ctxT[:, dc, :],
                                 start=(dc == 0), stop=(dc == NC - 1))
            nc.vector.tensor_copy(out=interT[:, ec, :], in_=it)
        ops_t = ps_proj.tile([128, D], f32, tag="proj")
        ops = ops_t[:, 0:L]
        for e in range(NC):
            nc.tensor.matmul(ops, interT[:, e, :], kbT_sb[:, e, :],
                             start=(e == 0), stop=(e == NC - 1))
        nc.vector.tensor_scalar(out=out_sb, in0=ops, scalar1=qm_sb,
                                scalar2=bbil_sb, op0=Alu.mult, op1=Alu.add)
        nc.sync.dma_start(out=out, in_=out_sb)


_CONSTS = None


def _consts():
    global _CONSTS
    if _CONSTS is None:
        import ml_dtypes
        _CONSTS = {
            "ident": np.eye(128, dtype=np.float32).astype(ml_dtypes.bfloat16),
        }
    return _CONSTS


_NC_CACHE = None


def _get_nc():
    global _NC_CACHE
    if _NC_CACHE is None:
        _NC_CACHE = build_nc()
    return _NC_CACHE


def _bf(x):
    import ml_dtypes
    return np.ascontiguousarray(x).astype(ml_dtypes.bfloat16)


def _pack_T(x, free):
    """[rows, D] -> transposed chunk-major [128, NC, rows] (bf16)."""
    import ml_dtypes
    xT = np.ascontiguousarray(x.T)  # [D, rows]
    return np.ascontiguousarray(
        xT.reshape(NC, 128, free).transpose(1, 0, 2)).astype(ml_dtypes.bfloat16)


def _pack_W(w):
    """[D, D] -> chunk-major [128, NC, D] (bf16)."""
    import ml_dtypes
    return np.ascontiguousarray(
        w.reshape(NC, 128, D).transpose(1, 0, 2)).astype(ml_dtypes.bfloat16)


def make_in_maps(queries, keys, values, Wq, bq, Wk, bk, Wv, bv, W_bil, b_bil):
    c = _consts()
    f = lambda x: np.asarray(x, dtype=np.float32)
    queries, keys, values = f(queries), f(keys), f(values)
    cstm = np.zeros((128, 2 * NC + 2), np.float32)
    cstm[:, 0:NC] = f(bq).reshape(NC, 128).T
    cstm[:, NC:2 * NC] = f(bk).reshape(NC, 128).T
    cstm[:, 2 * NC + 1] = f(b_bil)[0]
    rowsm = np.zeros((1, 128 + D), np.float32)
    rowsm[0, 0:128] = 1.0
    rowsm[0, 128:] = f(bv)
    shared = {
        "Wq": _pack_W(f(Wq)), "Wk": _pack_W(f(Wk)), "Wv": _pack_W(f(Wv)),
        "Wb": _pack_W(f(W_bil)),
        "rows": _bf(rowsm), "ident": c["ident"],
    }
    qmask = (np.abs(queries.sum(-1)) != 0.0).astype(np.float32)  # (B, L)
    in_maps = []
    for core in range(N_CORES):
        b, qh = divmod(core, 2)
        rows = slice(qh * LQ, (qh + 1) * LQ)
        m = dict(shared)
        m["qsT"] = _pack_T(queries[b, rows, :], LQ)
        m["kbT"] = _pack_T(keys[b], L)
        m["vbT"] = _pack_T(values[b], L)
        cm = cstm.copy()
        cm[:, 2 * NC] = qmask[b, rows]
        m["cst"] = cm
        in_maps.append(m)
    return in_maps


def kernel(**inputs):
    _ensure_path()
    from concourse.bass_utils import run_bass_kernel_spmd

    nc = _get_nc()
    in_maps = make_in_maps(**inputs)
    trace = os.environ.get("KERNEL_TRACE", "0") == "1"
    res = run_bass_kernel_spmd(nc, in_maps, core_ids=list(range(N_CORES)),
                               trace=trace)
    if trace:
        kernel.last_result = res
    out = np.zeros((B, L, L), np.float32)
    for core in range(N_CORES):
        b, qh = divmod(core, 2)
        out[b, qh * LQ:(qh + 1) * LQ, :] = res.results[core]["out"]
    return out
